# revision 1
# baseline (speedup 1.0000x reference)
"""CrossAssetGNN forward on 8 Trainium2 cores, data-parallel over batch.

Key algebraic reductions vs the reference:
- Only the last 15 timesteps of x feed the output (causal dilated convs,
  receptive field 15, last-timestep readout) -> upload/compute 15/128 of x.
- The gather/scatter GAT over E=16770 random edges collapses to dense
  130x130 ops via a host-precomputed edge-count matrix C[dst,src]:
  every per-edge quantity depends only on (src,dst), so duplicate edges
  fold into integer multiplicities. The softmax max-subtraction cancels
  (up to the 1e-8 epsilon, ~1e-10 relative) and is dropped.
- Edge-weight MLP is evaluated densely for all (dst,src) pairs with the
  relu'd pre-activation block as the *stationary* matmul operand so each
  result column lands partition-parallel in PSUM.
"""
import json
import sys

sys.path.insert(0, "/opt/trn_rl_repo")

import numpy as np
from contextlib import ExitStack

import concourse.bass as bass
import concourse.tile as tile
from concourse import masks, mybir
from concourse.bass_utils import run_bass_kernel_spmd

f32 = mybir.dt.float32
AF = mybir.ActivationFunctionType
OP = mybir.AluOpType

B, A, AUX, T, DIN, H, ODIM = 32, 128, 2, 128, 64, 128, 3
N = A + AUX            # 130
NC_CORES = 8
NB = B // NC_CORES     # 4 graphs per core
W = 15                 # receptive field of the three causal convs
BN_EPS = 1e-5
NCOL = NB * N          # 520 node columns per core
JBLK = 26              # j's per dense edge-MLP block


# ---- walrus workaround: max 1 sync-wait command per instruction ------------
def _apply_sync_split_patch():
    if getattr(bass.Bass, "_sync_split_patched", False):
        return
    orig = bass.Bass.to_json_bytes

    def to_json_bytes(self, *a, **kw):
        m = json.loads(orig(self, *a, **kw))
        for f in m.get("functions", []):
            for blk in f.get("blocks", []):
                new = []
                for inst in blk.get("instructions", []):
                    si = inst.get("sync_info")
                    if (si and si.get("on_wait") and len(si["on_wait"]) > 1
                            and inst.get("engine") in
                            {"PE", "DVE", "Activation", "SP", "Pool"}):
                        waits = si["on_wait"]
                        for k, w in enumerate(waits[:-1]):
                            new.append({"engine": inst["engine"], "ins": [],
                                        "outs": [],
                                        "name": f"{inst['name']}-sw{k}",
                                        "opcode": "NoOp",
                                        "sync_info": {"on_update": [],
                                                      "on_wait": [w]}})
                        si["on_wait"] = waits[-1:]
                    new.append(inst)
                blk["instructions"] = new
        return json.dumps(m).encode()

    bass.Bass.to_json_bytes = to_json_bytes
    bass.Bass._sync_split_patched = True


def _bcast_ap(t, offset_elems, dims):
    """AP over SBUF tile t: partition dim + given free [step, count] dims."""
    return bass.AP(tensor=t.tensor, offset=t.offset + offset_elems,
                   ap=[list(t.ap[0])] + [list(d) for d in dims])


def _chunks(total, step):
    return [(s, min(step, total - s)) for s in range(0, total, step)]


def build_program():
    nc = bass.Bass("TRN2", target_bir_lowering=False, num_devices=NC_CORES)

    din = {}

    def d_in(name, shape):
        din[name] = nc.dram_tensor(name, list(shape), f32, kind="ExternalInput")
        return din[name]

    d_in("xt", [DIN, NCOL * W])
    d_in("W_embT", [DIN, H]); d_in("b_emb", [H, 1])
    d_in("cw_all", [H, 9 * H]); d_in("sc_all", [H, 3]); d_in("bi_all", [H, 3])
    d_in("W1aT", [H, H]); d_in("W1bT", [H, H]); d_in("b1", [H, 1])
    d_in("w2", [H, 1])
    d_in("CA", [128, N]); d_in("CB", [2, N])
    d_in("gWT", [H, 3 * H]); d_in("asrc", [H, 3]); d_in("adst", [H, 3])
    d_in("hW1T", [H, A * 64]); d_in("b1exp", [64, A * NB])
    d_in("hW2T", [64, A * ODIM]); d_in("b2exp", [ODIM, A * NB])
    d_in("b2ew", [1, 1])

    o_logits = nc.dram_tensor("logits", [ODIM, A * NB], f32, kind="ExternalOutput")
    o_probs = nc.dram_tensor("probs", [128, NB * ODIM], f32, kind="ExternalOutput")

    with tile.TileContext(nc) as tc:
        with ExitStack() as top:
            const = top.enter_context(tc.tile_pool(name="const", bufs=1))
            persist = top.enter_context(tc.tile_pool(name="persist", bufs=1))

            def load(name, shape):
                t = const.tile(list(shape), f32, name=f"c_{name}", tag=f"c_{name}")
                nc.sync.dma_start(out=t, in_=din[name][:, :])
                return t

            W_embT = load("W_embT", [DIN, H]); b_emb = load("b_emb", [H, 1])
            cw_all = load("cw_all", [H, 9 * H])
            sc_all = load("sc_all", [H, 3]); bi_all = load("bi_all", [H, 3])
            W1aT = load("W1aT", [H, H]); W1bT = load("W1bT", [H, H])
            b1 = load("b1", [H, 1]); w2 = load("w2", [H, 1])
            CAt = load("CA", [128, N]); CBt = load("CB", [2, N])
            gWT = load("gWT", [H, 3 * H])
            asrc = load("asrc", [H, 3]); adst = load("adst", [H, 3])
            hW1T = load("hW1T", [H, A * 64]); b1exp = load("b1exp", [64, A * NB])
            hW2T = load("hW2T", [64, A * ODIM]); b2exp = load("b2exp", [ODIM, A * NB])
            b2ap = din["b2ew"][:, :]
            b2col = const.tile([128, 1], f32)
            nc.sync.dma_start(out=b2col, in_=bass.AP(
                tensor=b2ap.tensor, offset=b2ap.offset, ap=[[0, 128], [1, 1]]))

            ident = const.tile([128, 128], f32)
            masks.make_identity(nc, ident[:, :])
            alpha02 = const.tile([128, 1], f32)
            nc.vector.memset(alpha02[:, :], 0.2)
            ones_row = const.tile([1, NCOL], f32)
            nc.vector.memset(ones_row[:, :], 1.0)

            feats = persist.tile([H, NCOL], f32)

            # ---------------- stage A: embed + 3 dilated causal convs -------
            with ExitStack() as sA:
                front = sA.enter_context(tc.tile_pool(name="front", bufs=1))
                psA = sA.enter_context(
                    tc.tile_pool(name="psA", bufs=3, space="PSUM"))

                xT = front.tile([DIN, NCOL * W], f32)
                nc.sync.dma_start(out=xT, in_=din["xt"][:, :])
                emb = front.tile([H, NCOL * W], f32)
                for s, ln in _chunks(NCOL * W, 512):
                    pe = psA.tile([128, 512], f32, tag="pe")
                    nc.tensor.matmul(pe[:, :ln], lhsT=W_embT[:, :],
                                     rhs=xT[:, s:s + ln], start=True, stop=True)
                    nc.scalar.activation(emb[:, s:s + ln], pe[:, :ln],
                                         AF.Identity, bias=b_emb[:, :])

                # conv layers: (out_len per block, in_len, dilation)
                l1 = front.tile([H, NCOL * 13], f32)
                l2 = front.tile([H, NCOL * 9], f32)
                convs = [(emb, W, 13, 1, 0, l1), (l1, 13, 9, 2, 1, l2),
                         (l2, 9, 1, 4, 2, feats)]
                for src, in_len, out_len, dil, li, dst in convs:
                    sv = src.rearrange("p (blk t) -> p blk t", t=in_len)
                    bpc = max(1, 507 // out_len)
                    for b0, nb in _chunks(NCOL, bpc):
                        pe = psA.tile([128, 512], f32, tag="pe")
                        w_cols = nb * out_len
                        for k in range(3):
                            rhs = sv[:, b0:b0 + nb,
                                     k * dil:k * dil + out_len]
                            nc.tensor.matmul(
                                pe[:, :w_cols],
                                lhsT=cw_all[:, (li * 3 + k) * H:(li * 3 + k + 1) * H],
                                rhs=rhs, start=(k == 0), stop=(k == 2))
                        nc.scalar.activation(
                            dst[:, b0 * out_len:b0 * out_len + w_cols],
                            pe[:, :w_cols], AF.Gelu,
                            bias=bi_all[:, li:li + 1], scale=sc_all[:, li:li + 1])

            # ---------------- stage B: dense edge-weight MLP ----------------
            ewA = [persist.tile([128, N], f32, name=f"ewA{b}", tag=f"ewA{b}")
                   for b in range(NB)]
            ewB = [persist.tile([2, N], f32, name=f"ewB{b}", tag=f"ewB{b}")
                   for b in range(NB)]
            with ExitStack() as sB:
                ewk = sB.enter_context(tc.tile_pool(name="ewk", bufs=3))
                psU = sB.enter_context(tc.tile_pool(name="psU", bufs=2, space="PSUM"))
                psE = sB.enter_context(tc.tile_pool(name="psE", bufs=2, space="PSUM"))

                Ut = persist.tile([H, NCOL], f32)
                Vt = persist.tile([H, NCOL], f32)
                for s, ln in _chunks(NCOL, 512):
                    pu = psU.tile([128, 512], f32, tag="uv")
                    nc.tensor.matmul(pu[:, :ln], lhsT=W1aT[:, :],
                                     rhs=feats[:, s:s + ln], start=True, stop=True)
                    nc.vector.tensor_copy(Ut[:, s:s + ln], pu[:, :ln])
                    pv = psU.tile([128, 512], f32, tag="uv")
                    nc.tensor.matmul(pv[:, :ln], lhsT=W1bT[:, :],
                                     rhs=feats[:, s:s + ln], start=True, stop=True)
                    nc.scalar.activation(Vt[:, s:s + ln], pv[:, :ln],
                                         AF.Identity, bias=b1[:, :])

                for b in range(NB):
                    pA = psE.tile([128, N], f32, tag="ewpsA")
                    pB = psE.tile([2, N], f32, tag="ewpsB")
                    for jb in range(N // JBLK):
                        R = ewk.tile([128, JBLK * N], f32, tag="R")
                        in0 = _bcast_ap(Ut, b * N + jb * JBLK, [[1, JBLK], [0, N]])
                        in1 = _bcast_ap(Vt, b * N, [[0, JBLK], [1, N]])
                        nc.vector.tensor_tensor(out=R[:, :], in0=in0, in1=in1,
                                                op=OP.add)
                        nc.scalar.activation(R[:, :], R[:, :], AF.Relu)
                        for jl in range(JBLK):
                            j = jb * JBLK + jl
                            nc.tensor.matmul(pA[:, j:j + 1],
                                             lhsT=R[:, jl * N:jl * N + 128],
                                             rhs=w2[:, :], start=True, stop=True)
                            nc.tensor.matmul(pB[:, j:j + 1],
                                             lhsT=R[:, jl * N + 128:jl * N + N],
                                             rhs=w2[:, :], start=True, stop=True)
                    nc.scalar.activation(ewA[b][:, :], pA[:, :], AF.Sigmoid,
                                         bias=b2col[:, :])
                    nc.scalar.activation(ewB[b][:, :], pB[:, :], AF.Sigmoid,
                                         bias=b2col[0:2, :])

            # ---------------- stage C: 3 dense GAT layers -------------------
            nfT = feats
            with ExitStack() as sC:
                gw = sC.enter_context(tc.tile_pool(name="gw", bufs=2))
                gps = sC.enter_context(tc.tile_pool(name="gps", bufs=1, space="PSUM"))
                gsq = sC.enter_context(tc.tile_pool(name="gsq", bufs=2, space="PSUM"))

                for li in range(3):
                    gW = gWT[:, li * H:(li + 1) * H]
                    hpT = gw.tile([H, NCOL], f32, tag="hpT")
                    for s, ln in _chunks(NCOL, 512):
                        ph = gps.tile([128, 512], f32, tag="big")
                        nc.tensor.matmul(ph[:, :ln], lhsT=gW, rhs=nfT[:, s:s + ln],
                                         start=True, stop=True)
                        nc.vector.tensor_copy(hpT[:, s:s + ln], ph[:, :ln])

                    as_sb = gw.tile([1, NCOL], f32, tag="as")
                    ad_sb = gw.tile([1, NCOL], f32, tag="ad")
                    for col, vec, dst in ((0, asrc, as_sb), (1, adst, ad_sb)):
                        pav = gsq.tile([1, NCOL], f32, tag="arow", bufs=1)
                        for s, ln in _chunks(NCOL, 512):
                            nc.tensor.matmul(pav[0:1, s:s + ln],
                                             lhsT=vec[:, li:li + 1],
                                             rhs=hpT[:, s:s + ln],
                                             start=True, stop=True)
                        nc.vector.tensor_copy(dst[:, :], pav[:, :])

                    R2 = gw.tile([2, NCOL], f32, tag="R2")
                    nc.vector.memset(R2[0:1, :], 1.0)
                    nc.sync.dma_start(out=R2[1:2, :], in_=as_sb[:, :])

                    hpA, hpB = [], []
                    for b in range(NB):
                        pn = gsq.tile([128, 128], f32, tag="sq")
                        nc.tensor.matmul(pn[:, :], lhsT=nfT[:, b * N:b * N + 128],
                                         rhs=gW, start=True, stop=True)
                        ha = gw.tile([128, H], f32, name=f"hpA{b}", tag=f"hpA{b}")
                        nc.vector.tensor_copy(ha[:, :], pn[:, :])
                        hpA.append(ha)
                        pn2 = gsq.tile([2, 128], f32, tag="tiny")
                        nc.tensor.matmul(pn2[:, :], lhsT=nfT[:, b * N + 128:b * N + N],
                                         rhs=gW, start=True, stop=True)
                        hb = gw.tile([2, H], f32, name=f"hpB{b}", tag=f"hpB{b}")
                        nc.vector.tensor_copy(hb[:, :], pn2[:, :])
                        hpB.append(hb)

                    nfT_next = gw.tile([H, NCOL], f32, tag="nfT")
                    for b in range(NB):
                        L2b = gw.tile([2, N], f32, tag="L2b")
                        nc.vector.tensor_copy(L2b[0:1, :], ad_sb[0:1, b * N:(b + 1) * N])
                        nc.sync.dma_start(out=L2b[1:2, :], in_=ones_row[0:1, 0:N])

                        pa = gsq.tile([128, N], f32, tag="sq")
                        nc.tensor.matmul(pa[:, :N], lhsT=L2b[:, 0:128],
                                         rhs=R2[:, b * N:(b + 1) * N],
                                         start=True, stop=True)
                        pb = gsq.tile([2, N], f32, tag="tiny")
                        nc.tensor.matmul(pb[:, :N], lhsT=L2b[:, 128:N],
                                         rhs=R2[:, b * N:(b + 1) * N],
                                         start=True, stop=True)

                        PA = gw.tile([128, N], f32, tag="PA")
                        PB = gw.tile([2, N], f32, tag="PB")
                        sA_ = gw.tile([128, 1], f32, tag="sA")
                        sB_ = gw.tile([2, 1], f32, tag="sB")
                        for (pp, ew, Ct, Pt, st, rows) in (
                                (pa, ewA[b], CAt, PA, sA_, 128),
                                (pb, ewB[b], CBt, PB, sB_, 2)):
                            t_ = gw.tile([rows, N], f32, tag=f"t{rows}")
                            nc.scalar.activation(t_[:, :], pp[:rows, :N], AF.Prelu,
                                                 alpha=alpha02[:rows, :])
                            z_ = gw.tile([rows, N], f32, tag=f"z{rows}")
                            nc.vector.tensor_tensor(out=z_[:, :], in0=t_[:, :],
                                                    in1=ew[:, :], op=OP.mult)
                            e_ = gw.tile([rows, N], f32, tag=f"e{rows}")
                            nc.scalar.activation(e_[:, :], z_[:, :], AF.Exp)
                            nc.vector.scalar_tensor_tensor(
                                out=Pt[:, :], in0=e_[:, :], scalar=1.0,
                                in1=Ct[:, :], op0=OP.mult, op1=OP.mult,
                                accum_out=st[:, :])

                        rA = gw.tile([128, 1], f32, tag="rA")
                        rAn = gw.tile([128, 1], f32, tag="rAn")
                        rB = gw.tile([2, 1], f32, tag="rB")
                        rBn = gw.tile([2, 1], f32, tag="rBn")
                        for st, rr, rn in ((sA_, rA, rAn), (sB_, rB, rBn)):
                            nc.vector.tensor_scalar_add(st[:, :], st[:, :], 1e-8)
                            nc.vector.reciprocal(rr[:, :], st[:, :])
                            nc.vector.tensor_scalar_mul(rn[:, :], rr[:, :], -1.0)

                        # transpose P -> PT (src-major) for the aggregation
                        PT = gw.tile([128, N], f32, tag="PT")
                        PT2 = gw.tile([2, N], f32, tag="PT2")
                        pt1 = gsq.tile([128, 128], f32, tag="sq")
                        nc.tensor.transpose(pt1[:, :], PA[:, 0:128], ident[:, :])
                        nc.vector.tensor_copy(PT[:, 0:128], pt1[:, :])
                        pt2 = gsq.tile([2, 128], f32, tag="tiny")
                        nc.tensor.transpose(pt2[:, :], PA[:, 128:N], ident[:, :])
                        nc.vector.tensor_copy(PT2[:, 0:128], pt2[:, :])
                        pt3 = gsq.tile([128, 2], f32, tag="col2", bufs=1)
                        nc.tensor.transpose(pt3[:, :], PB[:, 0:128], ident[0:2, 0:2])
                        nc.vector.tensor_copy(PT[:, 128:N], pt3[:, :])
                        pt4 = gsq.tile([2, 2], f32, tag="tiny")
                        nc.tensor.transpose(pt4[:, :], PB[:, 128:N], ident[0:2, 0:2])
                        nc.vector.tensor_copy(PT2[:, 128:N], pt4[:, :])

                        po = gsq.tile([128, H], f32, tag="sq")
                        nc.tensor.matmul(po[:, :], lhsT=PT[:, 0:128], rhs=hpA[b][:, :],
                                         start=True, stop=False)
                        nc.tensor.matmul(po[:, :], lhsT=PT2[:, 0:128], rhs=hpB[b][:, :],
                                         start=False, stop=True)
                        po2 = gsq.tile([2, H], f32, tag="tiny")
                        nc.tensor.matmul(po2[:, :], lhsT=PT[:, 128:N], rhs=hpA[b][:, :],
                                         start=True, stop=False)
                        nc.tensor.matmul(po2[:, :], lhsT=PT2[:, 128:N], rhs=hpB[b][:, :],
                                         start=False, stop=True)

                        # elu(out * r) eviction, then transpose back to feat-major
                        for (pp, rr, rn, rows, coff) in (
                                (po, rA, rAn, 128, 0), (po2, rB, rBn, 2, 128)):
                            pos = gw.tile([rows, H], f32, tag=f"pos{rows}")
                            nc.scalar.activation(pos[:, :], pp[:rows, :], AF.Relu,
                                                 scale=rr[:rows, :])
                            neg = gw.tile([rows, H], f32, tag=f"neg{rows}")
                            nc.scalar.activation(neg[:, :], pp[:rows, :], AF.Relu,
                                                 scale=rn[:rows, :])
                            ex = gw.tile([rows, H], f32, tag=f"ex{rows}")
                            nc.scalar.activation(ex[:, :], neg[:, :], AF.Exp,
                                                 scale=-1.0)
                            nf_ = gw.tile([rows, H], f32, tag=f"nf{rows}")
                            nc.vector.scalar_tensor_tensor(
                                out=nf_[:, :], in0=ex[:, :], scalar=1.0,
                                in1=pos[:, :], op0=OP.subtract, op1=OP.add)
                            if rows == 128:
                                ptb = gsq.tile([128, 128], f32, tag="sq")
                                nc.tensor.transpose(ptb[:, :], nf_[:, :], ident[:, :])
                                nc.vector.tensor_copy(
                                    nfT_next[:, b * N:b * N + 128], ptb[:, :])
                            else:
                                ptb = gsq.tile([128, 2], f32, tag="col2", bufs=1)
                                nc.tensor.transpose(ptb[:, :], nf_[:, :],
                                                    ident[0:2, 0:2])
                                nc.vector.tensor_copy(
                                    nfT_next[:, b * N + 128:b * N + N], ptb[:, :])
                    nfT = nfT_next

            # ---------------- stage D: per-asset heads + softmax ------------
            with ExitStack() as sD:
                hw = sD.enter_context(tc.tile_pool(name="hw", bufs=1))
                hps = sD.enter_context(tc.tile_pool(name="hps", bufs=1, space="PSUM"))
                hsq = sD.enter_context(tc.tile_pool(name="hsq", bufs=4, space="PSUM"))

                hid_ps = hps.tile([64, A * NB], f32, tag="hid")
                for a in range(A):
                    rhs = bass.AP(tensor=nfT.tensor, offset=nfT.offset + a,
                                  ap=[list(nfT.ap[0]), [N, NB]])
                    nc.tensor.matmul(hid_ps[:, a * NB:(a + 1) * NB],
                                     lhsT=hW1T[:, a * 64:(a + 1) * 64],
                                     rhs=rhs, start=True, stop=True)
                hid = hw.tile([64, A * NB], f32)
                nc.vector.tensor_tensor(out=hid[:, :], in0=hid_ps[:, :],
                                        in1=b1exp[:, :], op=OP.add)
                nc.scalar.activation(hid[:, :], hid[:, :], AF.Relu)

                log_ps = hps.tile([ODIM, A * NB], f32, tag="log")
                for a in range(A):
                    nc.tensor.matmul(log_ps[:, a * NB:(a + 1) * NB],
                                     lhsT=hW2T[:, a * ODIM:(a + 1) * ODIM],
                                     rhs=hid[:, a * NB:(a + 1) * NB],
                                     start=True, stop=True)
                logits = hw.tile([ODIM, A * NB], f32)
                nc.vector.tensor_tensor(out=logits[:, :], in0=log_ps[:, :],
                                        in1=b2exp[:, :], op=OP.add)
                nc.sync.dma_start(out=o_logits[:, :], in_=logits[:, :])

                # softmax over ODIM: transpose to (128, 4, 3), exp on eviction
                e_sb = hw.tile([128, NB * ODIM], f32)
                for c in range(NB):
                    pt = hsq.tile([128, ODIM], f32, tag="sm")
                    nc.tensor.transpose(pt[:, :], logits[:, c * 128:(c + 1) * 128],
                                        ident[0:ODIM, 0:ODIM])
                    nc.scalar.activation(e_sb[:, c * ODIM:(c + 1) * ODIM],
                                         pt[:, :], AF.Exp)
                s_sb = hw.tile([128, NB], f32)
                for c in range(NB):
                    nc.vector.tensor_tensor(out=s_sb[:, c:c + 1],
                                            in0=e_sb[:, c * ODIM:c * ODIM + 1],
                                            in1=e_sb[:, c * ODIM + 1:c * ODIM + 2],
                                            op=OP.add)
                    nc.vector.tensor_tensor(out=s_sb[:, c:c + 1],
                                            in0=s_sb[:, c:c + 1],
                                            in1=e_sb[:, c * ODIM + 2:c * ODIM + 3],
                                            op=OP.add)
                r_sb = hw.tile([128, NB], f32)
                nc.vector.reciprocal(r_sb[:, :], s_sb[:, :])
                probs = hw.tile([128, NB * ODIM], f32)
                r_b = _bcast_ap(r_sb, 0, [[1, NB], [0, ODIM]])
                nc.vector.tensor_tensor(out=probs[:, :], in0=e_sb[:, :],
                                        in1=r_b, op=OP.mult)
                nc.sync.dma_start(out=o_probs[:, :], in_=probs[:, :])

    return nc


def host_inputs(x, edge_index, W_emb, b_emb, conv_w, conv_b, bn_gamma, bn_beta,
                bn_mean, bn_var, gat_W, gat_a_src, gat_a_dst, ew_W1, ew_b1,
                ew_W2, ew_b2, head_W1, head_b1, head_W2, head_b2):
    """Per-core input dicts (host-side preprocessing)."""
    f = np.float32
    xs = np.asarray(x, f)[:, :, T - W:, :]                       # (B,N,15,64)
    xt = np.ascontiguousarray(np.transpose(xs, (3, 0, 1, 2)))    # (64,B,N,15)

    ei = np.asarray(edge_index)
    C = np.zeros((N, N), f)
    np.add.at(C, (ei[1].astype(np.int64), ei[0].astype(np.int64)), 1.0)

    inv = np.asarray(bn_gamma, f) / np.sqrt(np.asarray(bn_var, f) + BN_EPS)
    sc_all = inv.T.copy()                                        # (H,3)
    bi_all = ((np.asarray(conv_b, f) - np.asarray(bn_mean, f)) * inv
              + np.asarray(bn_beta, f)).T.copy()                 # (H,3)
    cw = np.asarray(conv_w, f)                                   # (3,H,H,3)
    cw_all = np.concatenate(
        [cw[i, :, :, k].T for i in range(3) for k in range(3)], axis=1)

    ew_W1 = np.asarray(ew_W1, f)
    gat_W = np.asarray(gat_W, f)
    hW1 = np.asarray(head_W1, f); hW2 = np.asarray(head_W2, f)
    # b1exp[k, a*NB+bi] = head_b1[a,k]
    b1exp = np.repeat(np.asarray(head_b1, f).T[:, :, None], NB, axis=2)
    b1exp = b1exp.reshape(64, A * NB)
    b2exp = np.repeat(np.asarray(head_b2, f).T[:, :, None], NB, axis=2)
    b2exp = b2exp.reshape(ODIM, A * NB)

    shared = {
        "W_embT": np.ascontiguousarray(np.asarray(W_emb, f).T),
        "b_emb": np.asarray(b_emb, f).reshape(H, 1),
        "cw_all": np.ascontiguousarray(cw_all),
        "sc_all": np.ascontiguousarray(sc_all),
        "bi_all": np.ascontiguousarray(bi_all),
        "W1aT": np.ascontiguousarray(ew_W1[:, :H].T),
        "W1bT": np.ascontiguousarray(ew_W1[:, H:].T),
        "b1": np.asarray(ew_b1, f).reshape(H, 1),
        "w2": np.ascontiguousarray(np.asarray(ew_W2, f).reshape(1, H).T),
        "b2ew": np.asarray(ew_b2, f).reshape(1, 1),
        "CA": np.ascontiguousarray(C[:128]),
        "CB": np.ascontiguousarray(C[128:]),
        "gWT": np.ascontiguousarray(
            np.concatenate([gat_W[i].T for i in range(3)], axis=1)),
        "asrc": np.ascontiguousarray(
            np.stack([np.asarray(gat_a_src, f)[i, 0] for i in range(3)], axis=1)),
        "adst": np.ascontiguousarray(
            np.stack([np.asarray(gat_a_dst, f)[i, 0] for i in range(3)], axis=1)),
        "hW1T": np.ascontiguousarray(
            np.concatenate([hW1[a].T for a in range(A)], axis=1)),
        "b1exp": np.ascontiguousarray(b1exp),
        "hW2T": np.ascontiguousarray(
            np.concatenate([hW2[a].T for a in range(A)], axis=1)),
        "b2exp": np.ascontiguousarray(b2exp),
    }
    in_maps = []
    for c in range(NC_CORES):
        m = dict(shared)
        m["xt"] = np.ascontiguousarray(
            xt[:, c * NB:(c + 1) * NB].reshape(DIN, NCOL * W))
        in_maps.append(m)
    return in_maps


_CACHE = {}


def kernel(**inputs):
    _apply_sync_split_patch()
    if "nc" not in _CACHE:
        _CACHE["nc"] = build_program()
    nc = _CACHE["nc"]
    in_maps = host_inputs(**inputs)
    res = run_bass_kernel_spmd(nc, in_maps, list(range(NC_CORES)), trace=False)
    logits = np.empty((B, A, ODIM), np.float32)
    probs = np.empty((B, A, ODIM), np.float32)
    for c in range(NC_CORES):
        lg = res.results[c]["logits"]          # (3, A*NB)
        pr = res.results[c]["probs"]           # (128, NB*3)
        logits[c * NB:(c + 1) * NB] = lg.reshape(ODIM, A, NB).transpose(2, 1, 0)
        # probs rows: chunk c2 covers logit cols c2*128..; col idx = a*NB+bi
        tmp = pr.reshape(128, NB, ODIM).transpose(1, 0, 2).reshape(A * NB, ODIM)
        probs[c * NB:(c + 1) * NB] = tmp.reshape(A, NB, ODIM).transpose(1, 0, 2)
    return logits, probs



# revision 9
# speedup vs baseline: 2.3627x; 2.3627x over previous
"""CrossAssetGNN forward on 8 Trainium2 cores, data-parallel over batch.

bf16 rewrite of the fp32 baseline (653us). Key structural changes:
- All matmuls/transposes run in bf16 (1 cyc/row vs 4 for fp32, single
  hardware pass instead of two, cheaper LDWEIGHTS).
- The node-embedding matmul is folded into conv1 on the host:
  conv1(emb(x)) = sum_k (C1k @ W_emb) @ x_shift_k, so the kernel starts
  straight from the 15-timestep x slice.
- Edge-weight MLP: R blocks are built src-major ((v,g,j) col order) by
  DVE+GpSimd halves (outer-sum via stride-0 broadcast APs), relu on the
  Act engine, then 520 bf16 matmuls (R-block stationary, w2 streaming)
  produce ew^T[src, (g,dst)] directly; the 2 aux-j rows collapse to four
  [1,260] row-matmuls plus 2 partition-shifting DMAs.
- GAT layers work on the TRANSPOSED attention matrix: alpha^T is built
  by PE rank-2 matmuls, the exp/count chain multiplies by C^T (host
  upload), so P^T (the aggregation stationary) appears with NO on-device
  transposes of P; row sums come from a ones-vector matmul; only the
  nf node-major -> feature-major transpose remains (2 per graph).
- Per-asset heads: 2-asset-packed stationaries with zero-padded
  block-diagonal W2, junk quadrants killed by the zero blocks.
"""
import json
import sys

sys.path.insert(0, "/opt/trn_rl_repo")

import numpy as np
import ml_dtypes
from contextlib import ExitStack

import concourse.bass as bass
import concourse.tile as tile
from concourse import masks, mybir
from concourse.bass_utils import run_bass_kernel_spmd

f32 = mybir.dt.float32
bf16 = mybir.dt.bfloat16
AF = mybir.ActivationFunctionType
OP = mybir.AluOpType
BF = ml_dtypes.bfloat16

B, A, AUX, T, DIN, H, ODIM = 32, 128, 2, 128, 64, 128, 3
N = A + AUX            # 130
NC_CORES = 8
NB = B // NC_CORES     # 4 graphs per core
W = 15                 # receptive field of the three causal convs
BN_EPS = 1e-5
NCOL = NB * N          # 520 node columns per core


# ---- walrus workaround: max 1 sync-wait command per instruction ------------
def _apply_sync_split_patch():
    if getattr(bass.Bass, "_sync_split_patched", False):
        return
    orig = bass.Bass.to_json_bytes

    def to_json_bytes(self, *a, **kw):
        m = json.loads(orig(self, *a, **kw))
        for f in m.get("functions", []):
            for blk in f.get("blocks", []):
                new = []
                for inst in blk.get("instructions", []):
                    si = inst.get("sync_info")
                    if (si and si.get("on_wait") and len(si["on_wait"]) > 1
                            and inst.get("engine") in
                            {"PE", "DVE", "Activation", "SP", "Pool"}):
                        waits = si["on_wait"]
                        for k, w in enumerate(waits[:-1]):
                            new.append({"engine": inst["engine"], "ins": [],
                                        "outs": [],
                                        "name": f"{inst['name']}-sw{k}",
                                        "opcode": "NoOp",
                                        "sync_info": {"on_update": [],
                                                      "on_wait": [w]}})
                        si["on_wait"] = waits[-1:]
                    new.append(inst)
                blk["instructions"] = new
        return json.dumps(m).encode()

    bass.Bass.to_json_bytes = to_json_bytes
    bass.Bass._sync_split_patched = True


def _ap(t, offset_elems, dims):
    """AP over tile t: partition dim + given free [step, count] dims."""
    return bass.AP(tensor=t.tensor, offset=t.offset + offset_elems,
                   ap=[list(t.ap[0])] + [list(d) for d in dims])


def _papp(t, p0, p1, offset_elems, dims):
    """AP with partition slice [p0:p1] + free dims."""
    base = t[p0:p1, 0:1]
    return bass.AP(tensor=base.tensor, offset=base.offset + offset_elems,
                   ap=[list(base.ap[0])] + [list(d) for d in dims])


def _chunks(total, step):
    return [(s, min(step, total - s)) for s in range(0, total, step)]


def build_program():
    nc = bass.Bass("TRN2", target_bir_lowering=False, num_devices=NC_CORES)

    din = {}

    def d_in(name, shape, dt=bf16):
        din[name] = nc.dram_tensor(name, list(shape), dt, kind="ExternalInput")
        return din[name]

    d_in("xt", [DIN, NCOL * W])
    d_in("c1wT", [DIN, 3 * H])          # conv1 taps folded with W_emb
    d_in("cwT", [H, 6 * H])             # conv2/conv3 taps
    d_in("sc_all", [H, 3], f32)
    d_in("bi_all", [H, 3], f32)
    d_in("W1aT", [H, H]); d_in("W1bT", [H, H])
    d_in("b1f", [H, 1], f32)
    d_in("w2b", [H, 1])
    d_in("b2ew", [1, 1], f32)
    d_in("CAT", [128, N]); d_in("CBT", [2, N])     # C^T rows
    d_in("gWT", [H, 3 * H])
    d_in("asrcb", [H, 3]); d_in("adstb", [H, 3])
    d_in("hW1T", [H, A * 64])
    d_in("b1exp3", [128, A * NB], f32)
    d_in("W2blk", [H, (A // 2) * 2 * ODIM])
    d_in("b2exp", [ODIM, A * NB], f32)

    o_logits = nc.dram_tensor("logits", [ODIM, A * NB], f32,
                              kind="ExternalOutput")
    o_probs = nc.dram_tensor("probs", [128, NB * ODIM], f32,
                             kind="ExternalOutput")

    with tile.TileContext(nc) as tc:
        with ExitStack() as top:
            const = top.enter_context(tc.tile_pool(name="const", bufs=1))
            persist = top.enter_context(tc.tile_pool(name="persist", bufs=1))

            def load(name, shape, dt=bf16):
                t = const.tile(list(shape), dt, name=f"c_{name}",
                               tag=f"c_{name}")
                nc.sync.dma_start(out=t, in_=din[name][:, :])
                return t

            c1wT = load("c1wT", [DIN, 3 * H])
            cwT = load("cwT", [H, 6 * H])
            sc_all = load("sc_all", [H, 3], f32)
            bi_all = load("bi_all", [H, 3], f32)
            W1aT = load("W1aT", [H, H]); W1bT = load("W1bT", [H, H])
            b1f = load("b1f", [H, 1], f32)
            w2b = load("w2b", [H, 1])
            CAT = load("CAT", [128, N]); CBT = load("CBT", [2, N])
            gWT = load("gWT", [H, 3 * H])
            asrcb = load("asrcb", [H, 3]); adstb = load("adstb", [H, 3])
            hW1T = load("hW1T", [H, A * 64])
            b1exp3 = load("b1exp3", [128, A * NB], f32)
            W2blk = load("W2blk", [H, (A // 2) * 2 * ODIM])
            b2exp = load("b2exp", [ODIM, A * NB], f32)

            b2ap = din["b2ew"][:, :]
            b2col = const.tile([128, 1], f32)
            nc.sync.dma_start(out=b2col, in_=bass.AP(
                tensor=b2ap.tensor, offset=b2ap.offset, ap=[[0, 128], [1, 1]]))

            identb = const.tile([128, 128], bf16)
            masks.make_identity(nc, identb[:, :])
            identf = const.tile([128, 128], f32)
            masks.make_identity(nc, identf[:, :])
            alpha02 = const.tile([128, 1], f32)
            nc.vector.memset(alpha02[:, :], 0.2)
            ones128b = const.tile([128, 1], bf16)
            nc.vector.memset(ones128b[:, :], 1.0)
            ones_row = const.tile([1, NCOL], bf16)
            nc.vector.memset(ones_row[:, :], 1.0)

            feats = persist.tile([H, NCOL], bf16, name="feats")
            Ut = persist.tile([H, NCOL], bf16, name="Ut")
            Vt = persist.tile([H, NCOL], bf16, name="Vt")
            ewT_sb = persist.tile([128, NCOL], bf16, name="ewT")
            ewT2 = persist.tile([2, NCOL], bf16, name="ewT2")
            nfT_a = persist.tile([H, NCOL], bf16, name="nfT_a")
            nfT_b = persist.tile([H, NCOL], bf16, name="nfT_b")

            # ---------------- stage A: fused embed+conv1, conv2, conv3 ------
            with ExitStack() as sA:
                front = sA.enter_context(tc.tile_pool(name="front", bufs=1))
                psA = sA.enter_context(
                    tc.tile_pool(name="psA", bufs=3, space="PSUM"))

                xT = front.tile([DIN, NCOL * W], bf16)
                nc.sync.dma_start(out=xT, in_=din["xt"][:, :])
                l1 = front.tile([H, NCOL * 13], bf16)
                l2 = front.tile([H, NCOL * 9], bf16)

                # (src, taps_tile, tap0, in_len, out_len, dil, li, dst)
                convs = [(xT, c1wT, 0, W, 13, 1, 0, l1),
                         (l1, cwT, 0, 13, 9, 2, 1, l2),
                         (l2, cwT, 3, 9, 1, 4, 2, feats)]
                for src, taps, tap0, in_len, out_len, dil, li, dst in convs:
                    sv = src.rearrange("p (blk t) -> p blk t", t=in_len)
                    bpc = max(1, 507 // out_len) if out_len > 1 else 512
                    bpc = min(bpc, NCOL)
                    for b0, nb in _chunks(NCOL, bpc):
                        pe = psA.tile([128, 512], f32, tag="pe")
                        w_cols = nb * out_len
                        for k in range(3):
                            rhs = sv[:, b0:b0 + nb,
                                     k * dil:k * dil + out_len]
                            nc.tensor.matmul(
                                pe[:, :w_cols],
                                lhsT=taps[:, (tap0 + k) * H:(tap0 + k + 1) * H],
                                rhs=rhs, start=(k == 0), stop=(k == 2))
                        nc.scalar.activation(
                            dst[:, b0 * out_len:b0 * out_len + w_cols],
                            pe[:, :w_cols], AF.Gelu,
                            bias=bi_all[:, li:li + 1], scale=sc_all[:, li:li + 1])

            # ---------------- stage B: dense edge-weight MLP (src-major) ----
            with ExitStack() as sB:
                ewk = sB.enter_context(tc.tile_pool(name="ewk", bufs=2))
                psU = sB.enter_context(
                    tc.tile_pool(name="psU", bufs=2, space="PSUM"))
                psE = sB.enter_context(
                    tc.tile_pool(name="psE", bufs=1, space="PSUM"))

                for s, ln in _chunks(NCOL, 512):
                    pu = psU.tile([128, 512], f32, tag="uv")
                    nc.tensor.matmul(pu[:, :ln], lhsT=W1aT[:, :],
                                     rhs=feats[:, s:s + ln], start=True,
                                     stop=True)
                    nc.vector.tensor_copy(Ut[:, s:s + ln], pu[:, :ln])
                    pv = psU.tile([128, 512], f32, tag="uv")
                    nc.tensor.matmul(pv[:, :ln], lhsT=W1bT[:, :],
                                     rhs=feats[:, s:s + ln], start=True,
                                     stop=True)
                    nc.scalar.activation(Vt[:, s:s + ln], pv[:, :ln],
                                         AF.Identity, bias=b1f[:, :])

                ewTPS = [psE.tile([128, N], f32, name=f"ewTPS{g}",
                                  tag=f"ewTPS{g}") for g in range(NB)]

                # R blocks over v (26 dst per block), cols = (vl, g, j)
                for vb in range(5):
                    R = ewk.tile([128, 26 * NCOL], bf16, tag="R")
                    # split outer-sum add between DVE and GpSimd (13 vl each)
                    for eng, v0 in ((nc.vector, 0), (nc.gpsimd, 13)):
                        eng.tensor_tensor(
                            out=R[:, v0 * NCOL:(v0 + 13) * NCOL],
                            in0=_ap(Vt, vb * 26 + v0,
                                    [[1, 13], [N, NB], [0, N]]),
                            in1=_ap(Ut, 0, [[0, 13], [N, NB], [1, N]]),
                            op=OP.add)
                    nc.scalar.activation(R[:, :], R[:, :], AF.Relu)
                    for vl in range(26):
                        v = vb * 26 + vl
                        for g in range(NB):
                            c0 = (vl * NB + g) * N
                            nc.tensor.matmul(
                                ewTPS[g][:, v:v + 1],
                                lhsT=R[:, c0:c0 + 128], rhs=w2b[:, :],
                                start=True, stop=True)

                # aux-j rows (src 128..129): cols (j2, g, v)
                Raux = ewk.tile([128, 2 * NCOL], bf16, tag="Raux")
                nc.vector.tensor_tensor(
                    out=Raux[:, :],
                    in0=_ap(Ut, 128, [[1, 2], [N, NB], [0, N]]),
                    in1=_ap(Vt, 0, [[0, 2], [N, NB], [1, N]]),
                    op=OP.add)
                nc.scalar.activation(Raux[:, :], Raux[:, :], AF.Relu)
                ewstage = ewk.tile([1, 2 * NCOL], bf16, tag="ewstage")
                for c in range(4):
                    pax = psU.tile([1, 260], f32, tag="aux")
                    nc.tensor.matmul(pax[0:1, :],
                                     lhsT=w2b[:, :],
                                     rhs=Raux[:, c * 260:(c + 1) * 260],
                                     start=True, stop=True)
                    nc.scalar.activation(ewstage[0:1, c * 260:(c + 1) * 260],
                                         pax[0:1, :], AF.Sigmoid,
                                         bias=b2col[0:1, :])
                nc.sync.dma_start(out=ewT2[0:1, :], in_=ewstage[0:1, 0:NCOL])
                nc.sync.dma_start(out=ewT2[1:2, :],
                                  in_=ewstage[0:1, NCOL:2 * NCOL])

                for g in range(NB):
                    nc.scalar.activation(ewT_sb[:, g * N:(g + 1) * N],
                                         ewTPS[g][:, :], AF.Sigmoid,
                                         bias=b2col[:, :])

            # ---------------- stage C: 3 GAT layers (transposed P) ----------
            nfT_cur = feats
            with ExitStack() as sC:
                gw = sC.enter_context(tc.tile_pool(name="gw", bufs=2))
                gps = sC.enter_context(
                    tc.tile_pool(name="gps", bufs=1, space="PSUM"))

                for li in range(3):
                    gW = gWT[:, li * H:(li + 1) * H]
                    nfT_next = nfT_a if li % 2 == 0 else nfT_b

                    hpT = gw.tile([H, NCOL], bf16, tag="hpT")
                    for s, ln in _chunks(NCOL, 512):
                        ph = gps.tile([128, 512], f32, tag="big", bufs=2)
                        nc.tensor.matmul(ph[:, :ln], lhsT=gW,
                                         rhs=nfT_cur[:, s:s + ln],
                                         start=True, stop=True)
                        nc.vector.tensor_copy(hpT[:, s:s + ln], ph[:, :ln])

                    as_row = gw.tile([1, NCOL], bf16, tag="as_row")
                    ad_row = gw.tile([1, NCOL], bf16, tag="ad_row")
                    for vec, dst in ((asrcb, as_row), (adstb, ad_row)):
                        for s, ln in _chunks(NCOL, 512):
                            pr = gps.tile([128, 512], f32, tag="big", bufs=2)
                            nc.tensor.matmul(pr[0:1, :ln],
                                             lhsT=vec[:, li:li + 1],
                                             rhs=hpT[:, s:s + ln],
                                             start=True, stop=True)
                            nc.vector.tensor_copy(dst[0:1, s:s + ln],
                                                  pr[0:1, :ln])

                    AS2 = gw.tile([2, NCOL], bf16, tag="AS2")
                    nc.vector.tensor_copy(AS2[0:1, :], as_row[0:1, :])
                    nc.sync.dma_start(out=AS2[1:2, :], in_=ones_row[0:1, :])
                    AD2 = gw.tile([2, NCOL], bf16, tag="AD2")
                    nc.vector.tensor_copy(AD2[0:1, :], ones_row[0:1, :])
                    nc.sync.dma_start(out=AD2[1:2, :], in_=ad_row[0:1, :])

                    tT = gw.tile([128, NCOL], bf16, tag="tT")
                    tT2 = gw.tile([2, NCOL], bf16, tag="tT2")
                    for g in range(NB):
                        pac = gps.tile([128, 264], f32, tag="pa", bufs=2)
                        nc.tensor.matmul(pac[:, 0:N],
                                         lhsT=AS2[:, g * N:g * N + 128],
                                         rhs=AD2[:, g * N:(g + 1) * N],
                                         start=True, stop=True)
                        nc.scalar.activation(tT[:, g * N:(g + 1) * N],
                                             pac[:, 0:N], AF.Prelu,
                                             alpha=alpha02[:, :])
                        nc.tensor.matmul(pac[0:2, N:2 * N],
                                         lhsT=AS2[:, g * N + 128:(g + 1) * N],
                                         rhs=AD2[:, g * N:(g + 1) * N],
                                         start=True, stop=True)
                        nc.scalar.activation(tT2[0:2, g * N:(g + 1) * N],
                                             pac[0:2, N:2 * N], AF.Prelu,
                                             alpha=alpha02[0:2, :])

                    zT = gw.tile([128, NCOL], bf16, tag="zT")
                    nc.vector.tensor_tensor(out=zT[:, :], in0=tT[:, :],
                                            in1=ewT_sb[:, :], op=OP.mult)
                    zT2 = gw.tile([2, NCOL], bf16, tag="zT2")
                    nc.vector.tensor_tensor(out=zT2[:, :], in0=tT2[:, :],
                                            in1=ewT2[:, :], op=OP.mult)
                    eT = gw.tile([128, NCOL], bf16, tag="eT")
                    nc.scalar.activation(eT[:, :], zT[:, :], AF.Exp)
                    eT2 = gw.tile([2, NCOL], bf16, tag="eT2")
                    nc.scalar.activation(eT2[:, :], zT2[:, :], AF.Exp)
                    PT = gw.tile([128, NCOL], bf16, tag="PT")
                    nc.vector.tensor_tensor(
                        out=PT[:, :], in0=eT[:, :],
                        in1=_ap(CAT, 0, [[0, NB], [1, N]]), op=OP.mult)
                    PT2 = gw.tile([2, NCOL], bf16, tag="PT2")
                    nc.vector.tensor_tensor(
                        out=PT2[:, :], in0=eT2[:, :],
                        in1=_ap(CBT, 0, [[0, NB], [1, N]]), op=OP.mult)

                    # row sums over all 130 src via ones-matmul
                    sums_row = gw.tile([1, NCOL], f32, tag="sums")
                    for s, ln in _chunks(NCOL, 512):
                        pr = gps.tile([128, 512], f32, tag="big", bufs=2)
                        nc.tensor.matmul(pr[0:1, :ln], lhsT=ones128b[:, :],
                                         rhs=PT[:, s:s + ln],
                                         start=True, stop=False)
                        nc.tensor.matmul(pr[0:1, :ln], lhsT=ones128b[0:2, :],
                                         rhs=PT2[:, s:s + ln],
                                         start=False, stop=True)
                        nc.vector.tensor_scalar_add(sums_row[0:1, s:s + ln],
                                                    pr[0:1, :ln], 1e-8)
                    recip_row = gw.tile([1, NCOL], f32, tag="recip")
                    nc.vector.reciprocal(recip_row[0:1, :], sums_row[0:1, :])

                    rA = gw.tile([128, NB], f32, tag="rA")
                    rB = gw.tile([2, NB], f32, tag="rB")
                    for g in range(NB):
                        ptc = gps.tile([128, 264], f32, tag="pa", bufs=2)
                        nc.tensor.transpose(
                            ptc[:, 260:261], recip_row[0:1, g * N:g * N + 128],
                            identf[0:1, 0:1])
                        nc.vector.tensor_copy(rA[:, g:g + 1], ptc[:, 260:261])
                        nc.tensor.transpose(
                            ptc[0:2, 262:263],
                            recip_row[0:1, g * N + 128:(g + 1) * N],
                            identf[0:1, 0:1])
                        nc.vector.tensor_copy(rB[:, g:g + 1],
                                              ptc[0:2, 262:263])
                    rAn = gw.tile([128, NB], f32, tag="rAn")
                    nc.vector.tensor_scalar_mul(rAn[:, :], rA[:, :], -1.0)
                    rBn = gw.tile([2, NB], f32, tag="rBn")
                    nc.vector.tensor_scalar_mul(rBn[:, :], rB[:, :], -1.0)

                    for g in range(NB):
                        sq = gps.tile([128, 512], f32, tag="sq", bufs=2)
                        nc.tensor.matmul(sq[:, 0:H],
                                         lhsT=nfT_cur[:, g * N:g * N + 128],
                                         rhs=gW, start=True, stop=True)
                        hpA = gw.tile([128, H], bf16, tag="hpA")
                        nc.vector.tensor_copy(hpA[:, :], sq[:, 0:H])
                        nc.tensor.matmul(sq[0:2, H:2 * H],
                                         lhsT=nfT_cur[:, g * N + 128:(g + 1) * N],
                                         rhs=gW, start=True, stop=True)
                        hpB = gw.tile([2, H], bf16, tag="hpB")
                        nc.vector.tensor_copy(hpB[:, :], sq[0:2, H:2 * H])

                        nc.tensor.matmul(sq[:, 2 * H:3 * H],
                                         lhsT=PT[:, g * N:g * N + 128],
                                         rhs=hpA[:, :], start=True, stop=False)
                        nc.tensor.matmul(sq[:, 2 * H:3 * H],
                                         lhsT=PT2[:, g * N:g * N + 128],
                                         rhs=hpB[:, :], start=False, stop=True)
                        pos = gw.tile([128, H], bf16, tag="pos")
                        nc.scalar.activation(pos[:, :], sq[:, 2 * H:3 * H],
                                             AF.Relu, scale=rA[:, g:g + 1])
                        neg = gw.tile([128, H], bf16, tag="neg")
                        nc.scalar.activation(neg[:, :], sq[:, 2 * H:3 * H],
                                             AF.Relu, scale=rAn[:, g:g + 1])
                        ex = gw.tile([128, H], bf16, tag="ex")
                        nc.scalar.activation(ex[:, :], neg[:, :], AF.Exp,
                                             scale=-1.0)
                        nfnm = gw.tile([128, H], bf16, tag="nfnm")
                        nc.vector.scalar_tensor_tensor(
                            out=nfnm[:, :], in0=ex[:, :], scalar=1.0,
                            in1=pos[:, :], op0=OP.subtract, op1=OP.add)
                        ptb = gps.tile([128, 130], bf16, tag="tb", bufs=2)
                        nc.tensor.transpose(ptb[:, 0:128], nfnm[:, :],
                                            identb[:, :])
                        nc.vector.tensor_copy(nfT_next[:, g * N:g * N + 128],
                                              ptb[:, 0:128])

                        if li < 2:
                            nc.tensor.matmul(
                                sq[0:2, 3 * H:4 * H],
                                lhsT=PT[:, g * N + 128:(g + 1) * N],
                                rhs=hpA[:, :], start=True, stop=False)
                            nc.tensor.matmul(
                                sq[0:2, 3 * H:4 * H],
                                lhsT=PT2[:, g * N + 128:(g + 1) * N],
                                rhs=hpB[:, :], start=False, stop=True)
                            pos2 = gw.tile([2, H], bf16, tag="pos2")
                            nc.scalar.activation(pos2[:, :],
                                                 sq[0:2, 3 * H:4 * H],
                                                 AF.Relu, scale=rB[:, g:g + 1])
                            neg2 = gw.tile([2, H], bf16, tag="neg2")
                            nc.scalar.activation(neg2[:, :],
                                                 sq[0:2, 3 * H:4 * H],
                                                 AF.Relu,
                                                 scale=rBn[:, g:g + 1])
                            ex2 = gw.tile([2, H], bf16, tag="ex2")
                            nc.scalar.activation(ex2[:, :], neg2[:, :],
                                                 AF.Exp, scale=-1.0)
                            nfnm2 = gw.tile([2, H], bf16, tag="nfnm2")
                            nc.vector.scalar_tensor_tensor(
                                out=nfnm2[:, :], in0=ex2[:, :], scalar=1.0,
                                in1=pos2[:, :], op0=OP.subtract, op1=OP.add)
                            nc.tensor.transpose(ptb[:, 128:130], nfnm2[:, :],
                                                identb[0:2, 0:2])
                            nc.vector.tensor_copy(
                                nfT_next[:, g * N + 128:(g + 1) * N],
                                ptb[:, 128:130])
                    nfT_cur = nfT_next

            # ---------------- stage D: packed per-asset heads + softmax -----
            with ExitStack() as sD:
                hw = sD.enter_context(tc.tile_pool(name="hw", bufs=1))
                hps = sD.enter_context(
                    tc.tile_pool(name="hps", bufs=1, space="PSUM"))

                hid_ps = hps.tile([128, A * NB], f32, tag="hid")
                for p in range(A // 2):
                    nc.tensor.matmul(
                        hid_ps[:, p * 8:(p + 1) * 8],
                        lhsT=hW1T[:, p * 128:(p + 1) * 128],
                        rhs=_ap(nfT_cur, 2 * p, [[1, 2], [N, NB]]),
                        start=True, stop=True)
                hid_t = hw.tile([128, A * NB], bf16, tag="hid_t")
                nc.vector.tensor_tensor(out=hid_t[:, :], in0=hid_ps[:, :],
                                        in1=b1exp3[:, :], op=OP.add)
                hid3 = hw.tile([128, A * NB], bf16, tag="hid3")
                nc.scalar.activation(hid3[:, :], hid_t[:, :], AF.Relu)

                log_ps = hps.tile([2 * ODIM, A * NB], f32, tag="log")
                for p in range(A // 2):
                    nc.tensor.matmul(
                        log_ps[:, p * 8:(p + 1) * 8],
                        lhsT=W2blk[:, p * 6:(p + 1) * 6],
                        rhs=hid3[:, p * 8:(p + 1) * 8],
                        start=True, stop=True)
                stage6 = hw.tile([2 * ODIM, A * NB], f32, tag="stage6")
                nc.scalar.activation(stage6[:, :], log_ps[:, :], AF.Identity)
                logits = hw.tile([ODIM, A * NB], f32, tag="logits")
                # even assets: rows 0:3 at cols 8p..8p+4 (== a*4+b)
                nc.vector.tensor_copy(
                    _papp(logits, 0, 3, 0, [[8, 64], [1, 4]]),
                    _papp(stage6, 0, 3, 0, [[8, 64], [1, 4]]))
                # odd assets: rows 3:6 -> partition shift via DMA
                nc.sync.dma_start(
                    out=_papp(logits, 0, 3, 4, [[8, 64], [1, 4]]),
                    in_=_papp(stage6, 3, 6, 4, [[8, 64], [1, 4]]))
                nc.vector.tensor_tensor(out=logits[:, :], in0=logits[:, :],
                                        in1=b2exp[:, :], op=OP.add)
                nc.sync.dma_start(out=o_logits[:, :], in_=logits[:, :])

                # softmax over ODIM: transpose to (128, 4, 3), exp on eviction
                e_sb = hw.tile([128, NB * ODIM], f32, tag="e_sb")
                for c in range(NB):
                    pt = hps.tile([128, ODIM], f32, tag="sm", bufs=2)
                    nc.tensor.transpose(pt[:, :],
                                        logits[:, c * 128:(c + 1) * 128],
                                        identf[0:ODIM, 0:ODIM])
                    nc.scalar.activation(e_sb[:, c * ODIM:(c + 1) * ODIM],
                                         pt[:, :], AF.Exp)
                s_sb = hw.tile([128, NB], f32, tag="s_sb")
                for c in range(NB):
                    nc.vector.tensor_tensor(out=s_sb[:, c:c + 1],
                                            in0=e_sb[:, c * ODIM:c * ODIM + 1],
                                            in1=e_sb[:, c * ODIM + 1:c * ODIM + 2],
                                            op=OP.add)
                    nc.vector.tensor_tensor(out=s_sb[:, c:c + 1],
                                            in0=s_sb[:, c:c + 1],
                                            in1=e_sb[:, c * ODIM + 2:c * ODIM + 3],
                                            op=OP.add)
                r_sb = hw.tile([128, NB], f32, tag="r_sb")
                nc.vector.reciprocal(r_sb[:, :], s_sb[:, :])
                probs = hw.tile([128, NB * ODIM], f32, tag="probs")
                nc.vector.tensor_tensor(
                    out=probs[:, :], in0=e_sb[:, :],
                    in1=_ap(r_sb, 0, [[1, NB], [0, ODIM]]), op=OP.mult)
                nc.sync.dma_start(out=o_probs[:, :], in_=probs[:, :])

    return nc


def host_inputs(x, edge_index, W_emb, b_emb, conv_w, conv_b, bn_gamma, bn_beta,
                bn_mean, bn_var, gat_W, gat_a_src, gat_a_dst, ew_W1, ew_b1,
                ew_W2, ew_b2, head_W1, head_b1, head_W2, head_b2):
    """Per-core input dicts (host-side preprocessing)."""
    f = np.float32
    xs = np.asarray(x, f)[:, :, T - W:, :]                       # (B,N,15,64)
    xt = np.ascontiguousarray(np.transpose(xs, (3, 0, 1, 2)))    # (64,B,N,15)

    ei = np.asarray(edge_index)
    C = np.zeros((N, N), f)
    np.add.at(C, (ei[1].astype(np.int64), ei[0].astype(np.int64)), 1.0)
    CT = C.T.copy()                                              # [src, dst]

    cw = np.asarray(conv_w, f)                                   # (3,H,H,3)
    W_embf = np.asarray(W_emb, f)
    b_embf = np.asarray(b_emb, f)
    inv = np.asarray(bn_gamma, f) / np.sqrt(np.asarray(bn_var, f) + BN_EPS)
    sc_all = inv.T.copy()                                        # (H,3)
    cb_eff = np.asarray(conv_b, f).copy()
    cb_eff[0] = cb_eff[0] + cw[0].sum(axis=2) @ b_embf           # fold emb bias
    bi_all = ((cb_eff - np.asarray(bn_mean, f)) * inv
              + np.asarray(bn_beta, f)).T.copy()                 # (H,3)
    # conv1 taps folded with W_emb: (H,DIN) per tap; lhsT layout (DIN,H)
    c1wT = np.concatenate(
        [(cw[0, :, :, k] @ W_embf).T for k in range(3)], axis=1)  # (64,384)
    cwT = np.concatenate(
        [cw[i, :, :, k].T for i in (1, 2) for k in range(3)], axis=1)

    ew_W1 = np.asarray(ew_W1, f)
    gat_W = np.asarray(gat_W, f)
    hW1 = np.asarray(head_W1, f); hW2 = np.asarray(head_W2, f)
    hb1 = np.asarray(head_b1, f); hb2 = np.asarray(head_b2, f)

    # b1exp3[k-part, col=a*4+b]: rows 0:64 even-asset k, 64:128 odd-asset k
    b1exp3 = np.zeros((128, A * NB), f)
    for a in range(A):
        rows = slice(0, 64) if a % 2 == 0 else slice(64, 128)
        b1exp3[rows, a * NB:(a + 1) * NB] = hb1[a][:, None]
    # W2blk [128=(2a,64k), pair*6 + (2a,3o)] zero-padded block diagonal
    W2blk = np.zeros((H, (A // 2) * 2 * ODIM), f)
    for p in range(A // 2):
        W2blk[0:64, p * 6:p * 6 + 3] = hW2[2 * p].T           # (64k, 3o)
        W2blk[64:128, p * 6 + 3:p * 6 + 6] = hW2[2 * p + 1].T
    b2exp = np.repeat(hb2.T[:, :, None], NB, axis=2).reshape(ODIM, A * NB)

    bf = lambda a: np.ascontiguousarray(a).astype(BF)
    shared = {
        "c1wT": bf(c1wT),
        "cwT": bf(cwT),
        "sc_all": np.ascontiguousarray(sc_all),
        "bi_all": np.ascontiguousarray(bi_all),
        "W1aT": bf(ew_W1[:, :H].T),
        "W1bT": bf(ew_W1[:, H:].T),
        "b1f": np.asarray(ew_b1, f).reshape(H, 1),
        "w2b": bf(np.asarray(ew_W2, f).reshape(1, H).T),
        "b2ew": np.asarray(ew_b2, f).reshape(1, 1),
        "CAT": bf(CT[:128]),
        "CBT": bf(CT[128:]),
        "gWT": bf(np.concatenate([gat_W[i].T for i in range(3)], axis=1)),
        "asrcb": bf(np.stack([np.asarray(gat_a_src, f)[i, 0]
                              for i in range(3)], axis=1)),
        "adstb": bf(np.stack([np.asarray(gat_a_dst, f)[i, 0]
                              for i in range(3)], axis=1)),
        "hW1T": bf(np.concatenate([hW1[a].T for a in range(A)], axis=1)),
        "b1exp3": b1exp3,
        "W2blk": bf(W2blk),
        "b2exp": np.ascontiguousarray(b2exp),
    }
    in_maps = []
    for c in range(NC_CORES):
        m = dict(shared)
        m["xt"] = bf(xt[:, c * NB:(c + 1) * NB].reshape(DIN, NCOL * W))
        in_maps.append(m)
    return in_maps


_CACHE = {}


def kernel(**inputs):
    _apply_sync_split_patch()
    if "nc" not in _CACHE:
        _CACHE["nc"] = build_program()
    nc = _CACHE["nc"]
    in_maps = host_inputs(**inputs)
    res = run_bass_kernel_spmd(nc, in_maps, list(range(NC_CORES)), trace=False)
    logits = np.empty((B, A, ODIM), np.float32)
    probs = np.empty((B, A, ODIM), np.float32)
    for c in range(NC_CORES):
        lg = np.asarray(res.results[c]["logits"], np.float32)  # (3, A*NB)
        pr = np.asarray(res.results[c]["probs"], np.float32)   # (128, NB*3)
        logits[c * NB:(c + 1) * NB] = (
            lg.reshape(ODIM, A, NB).transpose(2, 1, 0))
        tmp = pr.reshape(128, NB, ODIM).transpose(1, 0, 2).reshape(A * NB, ODIM)
        probs[c * NB:(c + 1) * NB] = tmp.reshape(A, NB, ODIM).transpose(1, 0, 2)
    return logits, probs
